# revision 1
# baseline (speedup 1.0000x reference)
"""Trainium2 Bass kernel for a pre-norm transformer block (attention + MLP).

Contract: kernel(**inputs) takes the FULL unsharded inputs of
nn_Block_33775622815825 and returns the FULL output. The batch (B=128) is
sharded data-parallel across 8 NeuronCores (16 per core); the whole block
runs per-core with no collectives. All matmuls are bf16 with fp32 PSUM
accumulation (measured scale-relative error ~1.2e-3 vs the fp32 reference).

Per core, per batch element b (two-phase so bf16 weights fit in SBUF):
  phase A:  x -> LN1 -> h(bf16) -> hT chunks (batched DMA-xbar transpose)
            Q^T/K^T = Wqk^T @ hT  (two batch elems share each matmul, N=256)
            V = hT^T @ Wv          (interleaved into the scores pass below
                                    as PE filler while exps drain)
            per head pair: scoresT[s,t] = K_h Q_h^T/32 + (-1e9)*causal (PSUM)
                           weiT = exp(scoresT)      (one ACT op, bf16 out)
            per 4 heads:   attn_un[t,d] = weiT^T @ V_h   } same PSUM tile,
                           rowsum[t]    = weiT^T @ ones  } fused rowsum col
                           attn = attn_un * (1/rowsum)   (DVE, recip+bcast)
                           attnT block <- DMA-xbar transpose (proj layout)
            x1 = x + attnT^T @ Wproj + b_proj  (bias via rank-1 matmul,
                 residual via DRAM->DRAM seed + SWDGE accumulate-DMA)
  phase B:  x1 -> LN2 -> h2T
            hidT = relu(W1^T @ h2T + r1)   (bias fused into ACT relu drain)
            out  = x1 + hidT^T @ W2 + b2

Key tricks: LN gammas folded into weights host-side; LN betas / linear
biases become rank-1 matmul accumulations or per-partition ACT biases
(emitted only when nonzero); softmax skips max-subtraction (scores are
tiny); the causal mask is a -1e9 matmul into the scores PSUM; rstd is
exp(-0.5*ln(var+eps)) and a custom Bacc pass pins every activation to the
single natural_log_exp_and_others table set (one ACT_TABLE_LOAD total);
emission order software-pipelines the frontend one pair ahead and
interleaves next-batch work into the softmax tails to keep the in-order
PE queue fed.
"""
import os
import sys

import numpy as np

for _p in ("/opt/trn_rl_repo",):
    if _p not in sys.path:
        sys.path.insert(0, _p)

import ml_dtypes

import concourse.bass as bass
import concourse.tile as tile
from concourse import bacc
from concourse import mybir
from concourse.bass import ds, ts

F32 = mybir.dt.float32
BF16 = mybir.dt.bfloat16
AF = mybir.ActivationFunctionType
ALU = mybir.AluOpType

B, T, C, H, D = 128, 128, 1024, 16, 64
C4 = 4 * C
NCORES = 8
BPC = B // NCORES  # batch elements per core
EPS = 1e-5
SCL = float(C) ** -0.5  # softmax scale (1/32)
NEG = -1.0e9

bf16 = ml_dtypes.bfloat16


class _Bacc(bacc.Bacc):
    """Bacc whose activation-table pass resolves every function we use to
    the single `natural_log_exp_and_others` set (exp+ln+relu+copy live
    there together), so the kernel pays exactly one ACT_TABLE_LOAD instead
    of thrashing between exp_and_others and natural_log_exp_and_others."""

    _ONE_SET = "natural_log_exp_and_others"
    _OURS = {AF.Exp, AF.Ln, AF.Copy, AF.Relu, AF.Identity, AF.Square}

    def insert_act_table_loads(self):
        import bass_rust as _br

        from concourse.hw_specs import get_activation_tables

        has_activation = any(
            isinstance(i, mybir.InstActivation)
            for b in self.main_func.blocks
            for i in b.instructions
        )
        if not has_activation:
            return
        tables = []
        for name, funcs in get_activation_tables(self.m.arch).items():
            if name != self._ONE_SET:
                funcs = set(funcs) - self._OURS
            tables.append((name, funcs))
        _br.insert_act_table_loads(self, tables)


def build_program(bpc=BPC, qkv_bias=False, v_bias=False):
    """Emit the Bass/Tile program for one core handling `bpc` batch elems."""
    assert bpc % 2 == 0
    nc = _Bacc()

    xin = nc.declare_dram_parameter("xin", [bpc, T, C], F32, isOutput=False)
    # [c_part, {wq,wk,wv,wproj}, chunk, 1024] ; wproj is [hd_part, chunk, c]
    wa = nc.declare_dram_parameter("wa", [128, 4, 8, 1024], BF16, isOutput=False)
    w1 = nc.declare_dram_parameter("w1", [128, 8, C4], BF16, isOutput=False)
    w2 = nc.declare_dram_parameter("w2", [128, 32, C], BF16, isOutput=False)
    ident = nc.declare_dram_parameter("ident", [128, 128], BF16, isOutput=False)
    maskm = nc.declare_dram_parameter("maskm", [128, 128], BF16, isOutput=False)
    onesr = nc.declare_dram_parameter("onesr", [1, 256], BF16, isOutput=False)
    rows = nc.declare_dram_parameter("rows", [1, 2, C], BF16, isOutput=False)
    r1c = nc.declare_dram_parameter("r1c", [128, 32], F32, isOutput=False)
    if qkv_bias:
        qkr = nc.declare_dram_parameter("qkr", [1, 2, 8, 128], BF16, isOutput=False)
    if v_bias:
        vr = nc.declare_dram_parameter("vr", [1, C], BF16, isOutput=False)
    yout = nc.declare_dram_parameter("yout", [bpc, T, C], F32, isOutput=True)
    x1d = nc.dram_tensor("x1d", [bpc, T, C], F32)

    xin, wa, w1, w2 = xin[:], wa[:], w1[:], w2[:]
    ident_d, maskm_d, onesr_d, rows_d, r1c_d = (
        ident[:], maskm[:], onesr[:], rows[:], r1c[:],
    )
    yout, x1d = yout[:], x1d[:]

    npairs = bpc // 2

    with tile.TileContext(nc) as tc:
        from contextlib import ExitStack

        with ExitStack() as ctx:
            consts = ctx.enter_context(tc.tile_pool(name="consts", bufs=1))
            wpool = ctx.enter_context(tc.tile_pool(name="wpool", bufs=1))
            w1pool = ctx.enter_context(tc.tile_pool(name="w1pool", bufs=1))
            xpool = ctx.enter_context(tc.tile_pool(name="xpool", bufs=3))
            hpool = ctx.enter_context(tc.tile_pool(name="hpool", bufs=2))
            htpool = ctx.enter_context(tc.tile_pool(name="htpool", bufs=3))
            respool = ctx.enter_context(tc.tile_pool(name="respool", bufs=2))
            stats = ctx.enter_context(tc.tile_pool(name="stats", bufs=4))
            # PSUM: acc (QKV / mlp1 accumulations), scatt (scores + attnT),
            # wide (proj / mlp2 outputs). 2 + 4 + 2 = 8 banks.
            psacc = ctx.enter_context(
                tc.tile_pool(name="psacc", bufs=2, space="PSUM")
            )


            # constants
            ident_sb = consts.tile([128, 128], BF16)
            nc.gpsimd.dma_start(out=ident_sb, in_=ident_d)
            maskm_sb = consts.tile([128, 128], BF16)
            nc.gpsimd.dma_start(out=maskm_sb, in_=maskm_d)
            ones_sb = consts.tile([1, 256], BF16)
            nc.gpsimd.dma_start(out=ones_sb, in_=onesr_d)
            rows_sb = consts.tile([1, 2, C], BF16)
            nc.gpsimd.dma_start(out=rows_sb, in_=rows_d)
            r1_sb = consts.tile([128, 32], F32)
            nc.gpsimd.dma_start(out=r1_sb, in_=r1c_d)
            eps_sb = consts.tile([128, 1], F32)
            nc.vector.memset(eps_sb, EPS)
            ones_col = consts.tile([128, 1], BF16)
            nc.vector.memset(ones_col, 1.0)
            if qkv_bias:
                qkr_sb = consts.tile([1, 2, 8, 128], BF16)
                nc.gpsimd.dma_start(out=qkr_sb, in_=qkr[:])
            if v_bias:
                vr_sb = consts.tile([1, C], BF16)
                nc.gpsimd.dma_start(out=vr_sb, in_=vr[:])

            waA = wpool.tile([128, 4, 8, 1024], BF16, tag="wbig")
            w1t = w1pool.tile([128, 8, C4], BF16)

            def layernorm(x_sb, tagp):
                """LN stats on [128, C] fp32; returns bf16 normalized tile."""
                st6 = stats.tile([128, 2, 6], F32, tag="st6" + tagp)
                nc.vector.bn_stats(out=st6[:, 0], in_=x_sb[:, 0:512])
                nc.vector.bn_stats(out=st6[:, 1], in_=x_sb[:, 512:1024])
                mv = stats.tile([128, 2], F32, tag="mv" + tagp)
                nc.vector.bn_aggr(out=mv, in_=st6)
                # rstd = exp(-0.5*ln(var+eps)) : stays in the exp/ln table set
                lnv = stats.tile([128, 1], F32, tag="lnv" + tagp)
                nc.scalar.activation(out=lnv, in_=mv[:, 1:2], func=AF.Ln, bias=eps_sb)
                rstd = stats.tile([128, 1], F32, tag="rstd" + tagp)
                nc.scalar.activation(out=rstd, in_=lnv, func=AF.Exp, scale=-0.5)
                # bounce rstd through DVE so the (wait-slot-limited)
                # tensor_scalar below never carries a cross-engine wait
                rstd2 = stats.tile([128, 1], F32, tag="rstd2" + tagp)
                nc.vector.tensor_copy(out=rstd2, in_=rstd)
                h_sb = hpool.tile([128, C], BF16, tag="h")
                nc.vector.tensor_scalar(
                    out=h_sb, in0=x_sb, scalar1=mv[:, 0:1], scalar2=rstd2,
                    op0=ALU.subtract, op1=ALU.mult,
                )
                return h_sb

            def load_ln_transpose(b, hT, pi, src):
                x_sb = xpool.tile([128, C], F32, tag="x", name=f"x_{b}")
                nc.gpsimd.dma_start(out=x_sb, in_=src[b])
                h_sb = layernorm(x_sb, "a")
                nc.sync.dma_start(out=hT[:, pi], in_=h_sb, transpose=True)

            def residual_init(b, src, dst):
                # seed dst with the residual stream (DRAM->DRAM, off the
                # critical path); the matmul result is DMA-accumulated later
                nc.gpsimd.dma_start(out=dst[b], in_=src[b])

            def residual_store(b, psum, dst):
                o_sb = respool.tile([128, C], F32, tag="res", name=f"res_{b}")
                nc.scalar.copy(out=o_sb, in_=psum)
                nc.gpsimd.dma_start(out=dst[b], in_=o_sb,
                                    accum_op=ALU.add)

            # ---------------- phase A ----------------
            with ExitStack() as actx:
                psatt = actx.enter_context(
                    tc.tile_pool(name="psatt", bufs=2, space="PSUM")
                )
                pswide = actx.enter_context(
                    tc.tile_pool(name="pswide", bufs=1, space="PSUM")
                )
                qkpool = actx.enter_context(tc.tile_pool(name="qkpool", bufs=2))
                vpool = actx.enter_context(tc.tile_pool(name="vpool", bufs=2))
                apool = actx.enter_context(tc.tile_pool(name="apool", bufs=2))
                weipool = actx.enter_context(tc.tile_pool(name="weipool", bufs=4))

                def frontA(pair, inits=True):
                    hT = htpool.tile([128, 2, 8, 128], BF16, tag="ht",
                                     name=f"hT_{pair}")
                    for pi in range(2):
                        load_ln_transpose(2 * pair + pi, hT, pi, xin)
                        if inits:
                            residual_init(2 * pair + pi, xin, x1d)
                    return hT

                # wq/wk ride the scalar HWDGE queue, which is idle at
                # startup -- they land in parallel with the first x loads
                nc.scalar.dma_start(out=waA[:, 0], in_=wa[:, 0])
                nc.scalar.dma_start(out=waA[:, 1], in_=wa[:, 1])
                front_next = frontA(0, inits=False)
                for wi in range(2, 4):
                    nc.gpsimd.dma_start(out=waA[:, wi], in_=wa[:, wi])
                for pi in range(2):
                    residual_init(pi, xin, x1d)

                for pair in range(npairs):
                    hT = front_next
                    if pair + 1 < npairs:
                        front_next = frontA(pair + 1)
                    # stream w1 slabs during phase A
                    if pair < 8:
                        nc.gpsimd.dma_start(out=w1t[:, pair], in_=w1[:, pair])
                    if pair == npairs - 1:
                        for s in range(min(npairs, 8), 8):
                            nc.gpsimd.dma_start(out=w1t[:, s], in_=w1[:, s])

                    # Q^T / K^T : [hd', 2b, t] accumulated over c-chunks
                    qt_sb = qkpool.tile([128, 2, 8, 128], BF16, tag="qt")
                    kt_sb = qkpool.tile([128, 2, 8, 128], BF16, tag="kt")
                    for wi, dst in ((0, qt_sb), (1, kt_sb)):
                        for blk in range(8):
                            ps = psacc.tile([128, 2, 128], F32, tag="ps")
                            if qkv_bias:
                                nc.tensor.matmul(
                                    ps, lhsT=qkr_sb[0:1, wi, blk],
                                    rhs=ones_sb,
                                    start=True, stop=False,
                                )
                            for cc in range(8):
                                nc.tensor.matmul(
                                    ps,
                                    lhsT=waA[:, wi, cc, ts(blk, 128)],
                                    rhs=hT[:, :, cc],
                                    start=(cc == 0 and not qkv_bias),
                                    stop=(cc == 7),
                                )
                            nc.scalar.copy(out=dst[:, :, blk], in_=ps)
                    state = {}

                    def sc_pass(pi):
                        b = 2 * pair + pi
                        v_sb = vpool.tile([128, C], BF16, tag="v")

                        def v_half(half, pi=pi, v_sb=v_sb):
                            psv = psacc.tile([128, 512], F32, tag="ps")
                            if v_bias:
                                nc.tensor.matmul(
                                    psv, lhsT=ones_sb[0:1, 0:128],
                                    rhs=vr_sb[0:1, ds(half * 512, 512)],
                                    start=True, stop=False,
                                )
                            for cc in range(8):
                                nc.tensor.matmul(
                                    psv,
                                    lhsT=hT[:, pi, cc],
                                    rhs=waA[:, 2, cc, ds(half * 512, 512)],
                                    start=(cc == 0 and not v_bias),
                                    stop=(cc == 7),
                                )
                            nc.scalar.copy(
                                out=v_sb[:, ds(half * 512, 512)], in_=psv
                            )

                        # pass 1: transposed scores exp(K Q^T / 32) for all 16
                        # heads -- the exp output IS the attn matmul's
                        # stationary operand, so no wei transpose is needed.
                        # V matmuls interleave as PE filler.
                        weiTs = []
                        for blk in range(8):
                            weiT_un = weipool.tile(
                                [128, 2, 128], BF16, tag="weiT", bufs=10
                            )
                            # two heads' scoresT share one PSUM bank; one exp
                            sc2 = psatt.tile([128, 2, 128], F32, tag="ps")
                            for sub in range(2):
                                po = sub * 64
                                nc.tensor.matmul(
                                    sc2[:, sub], lhsT=kt_sb[po:po + 64, pi, blk],
                                    rhs=qt_sb[po:po + 64, pi, blk],
                                    start=True, stop=False,
                                )
                                nc.tensor.matmul(
                                    sc2[:, sub], lhsT=maskm_sb, rhs=ident_sb,
                                    start=False, stop=True,
                                )
                            nc.scalar.activation(
                                out=weiT_un, in_=sc2, func=AF.Exp, scale=SCL,
                            )
                            weiTs.append(weiT_un)
                            if blk == 2:
                                v_half(0)
                            elif blk == 5:
                                v_half(1)
                        state[pi] = (v_sb, weiTs)

                    def attnT_pass(pi):
                        v_sb, weiTs = state[pi]
                        attnT = apool.tile([128, 8, 128], BF16, tag="attnT")
                        for grp in range(4):  # 4 heads per group
                            # attn_un [t, head, d] plus a fused rowsum column
                            att_ps = psatt.tile([128, 4, 65], F32, tag="att",
                                                bufs=2)
                            for j in range(4):
                                hh = 4 * grp + j
                                blk, sub = hh // 2, hh % 2
                                nc.tensor.matmul(
                                    att_ps[:, j, 0:64],
                                    lhsT=weiTs[blk][:, sub],
                                    rhs=v_sb[:, ds(hh * 64, 64)],
                                    start=True, stop=True,
                                )
                                nc.tensor.matmul(
                                    att_ps[:, j, 64:65],
                                    lhsT=weiTs[blk][:, sub],
                                    rhs=ones_col,
                                    start=True, stop=True,
                                )
                            rr4 = stats.tile([128, 4], F32, tag="rr")
                            nc.vector.reciprocal(out=rr4, in_=att_ps[:, :, 64])
                            attn_bf = weipool.tile([128, 4, 64], BF16,
                                                   tag="anorm", bufs=4)
                            nc.vector.tensor_mul(
                                out=attn_bf, in0=att_ps[:, :, 0:64],
                                in1=rr4.to_broadcast([128, 4, 64]),
                            )
                            # blockwise xbar transpose writes attnT directly
                            # in the proj lhsT layout [hd', t]
                            nc.sync.dma_start(
                                out=attnT[:, ts(grp, 2)], in_=attn_bf,
                                transpose=True,
                            )
                        state[pi] = attnT

                    def proj_pass(pi):
                        attnT = state[pi]
                        b = 2 * pair + pi
                        prp = pswide.tile([128, C], F32, tag="wide")
                        for half in range(2):
                            nc.tensor.matmul(
                                prp[:, ds(half * 512, 512)],
                                lhsT=ones_sb[0:1, 0:128],
                                rhs=rows_sb[0:1, 0, ds(half * 512, 512)],
                                start=True, stop=False,
                            )
                            for hc in range(8):
                                nc.tensor.matmul(
                                    prp[:, ds(half * 512, 512)],
                                    lhsT=attnT[:, hc],
                                    rhs=waA[:, 3, hc, ds(half * 512, 512)],
                                    start=False, stop=(hc == 7),
                                )
                        residual_store(b, prp, x1d)

                    # emission order keeps PE fed through the b1 softmax
                    # tail: b0's proj fills the gap before b1's attnT
                    sc_pass(0)
                    attnT_pass(0)
                    sc_pass(1)
                    proj_pass(0)
                    attnT_pass(1)
                    proj_pass(1)

            # ---------------- phase B ----------------
            w2t = wpool.tile([128, 32, C], BF16, tag="wbig")
            for mc in range(4):
                nc.gpsimd.dma_start(out=w2t[:, ts(mc, 8)], in_=w2[:, ts(mc, 8)])

            with tc.tile_pool(name="hidpool", bufs=2) as hidpool, \
                    tc.tile_pool(name="pswideB", bufs=2,
                                 space="PSUM") as pswideB:

                def frontB(pair):
                    h2T = htpool.tile([128, 2, 8, 128], BF16, tag="ht",
                                      name=f"h2T_{pair}")
                    for pi in range(2):
                        load_ln_transpose(2 * pair + pi, h2T, pi, x1d)
                        residual_init(2 * pair + pi, x1d, yout)
                    return h2T

                front_next = frontB(0)
                for pair in range(npairs):
                    h2T = front_next
                    if pair + 1 < npairs:
                        front_next = frontB(pair + 1)
                    hid = hidpool.tile([128, 2, 32, 128], BF16, tag="hid")
                    for mb in range(32):
                        ps1 = psacc.tile([128, 2, 128], F32, tag="ps")
                        for cc in range(8):
                            nc.tensor.matmul(
                                ps1, lhsT=w1t[:, cc, ts(mb, 128)],
                                rhs=h2T[:, :, cc],
                                start=(cc == 0), stop=(cc == 7),
                            )
                        nc.scalar.activation(
                            out=hid[:, :, mb], in_=ps1, func=AF.Relu,
                            bias=r1_sb[:, mb:mb + 1],
                        )
                    for pi in range(2):
                        b = 2 * pair + pi
                        ps2 = pswideB.tile([128, C], F32, tag="wideB")
                        for half in range(2):
                            nc.tensor.matmul(
                                ps2[:, ds(half * 512, 512)],
                                lhsT=ones_sb[0:1, 0:128],
                                rhs=rows_sb[0:1, 1, ds(half * 512, 512)],
                                start=True, stop=False,
                            )
                            for mc in range(32):
                                nc.tensor.matmul(
                                    ps2[:, ds(half * 512, 512)],
                                    lhsT=hid[:, pi, mc],
                                    rhs=w2t[:, mc, ds(half * 512, 512)],
                                    start=False, stop=(mc == 31),
                                )
                        residual_store(b, ps2, yout)

    # lower to HW-legal IR: split >1-wait instructions into EventSemaphore
    # preludes, move matmul waits onto ldweights, alloc regs, act tables
    nc.compile()
    return nc


def prep_host(inputs):
    """Host-side weight packing / folding. Returns (shared in_map, flags)."""
    f32 = np.float32
    wq = np.asarray(inputs["wq"], f32)
    wk = np.asarray(inputs["wk"], f32)
    wv = np.asarray(inputs["wv"], f32)
    w_proj = np.asarray(inputs["w_proj"], f32)
    b_proj = np.asarray(inputs["b_proj"], f32)
    w1 = np.asarray(inputs["w1"], f32)
    b1 = np.asarray(inputs["b1"], f32)
    w2 = np.asarray(inputs["w2"], f32)
    b2 = np.asarray(inputs["b2"], f32)
    g1 = np.asarray(inputs["ln1_g"], f32)
    bt1 = np.asarray(inputs["ln1_b"], f32)
    g2 = np.asarray(inputs["ln2_g"], f32)
    bt2 = np.asarray(inputs["ln2_b"], f32)

    wq_f = wq.transpose(1, 0, 2).reshape(C, C)  # [c, h*d]
    wk_f = wk.transpose(1, 0, 2).reshape(C, C)
    wv_f = wv.transpose(1, 0, 2).reshape(C, C)

    # fold LN1 gamma into qkv weights; LN1 beta becomes rank-1 rows
    rq = bt1 @ wq_f
    rk = bt1 @ wk_f
    rv = bt1 @ wv_f
    qkv_bias = bool(np.abs(rq).max() > 0 or np.abs(rk).max() > 0)
    v_bias = bool(np.abs(rv).max() > 0)

    wa = np.stack(
        [g1[:, None] * wq_f, g1[:, None] * wk_f, g1[:, None] * wv_f, w_proj], 0
    )  # [4, 1024, 1024]
    wa = wa.reshape(4, 8, 128, 1024).transpose(2, 0, 1, 3)  # [128, 4, 8, 1024]

    w1_eff = g2[:, None] * w1  # [C, 4C]
    w1p = w1_eff.reshape(8, 128, C4).transpose(1, 0, 2)  # [128, 8, 4C]
    w2p = w2.reshape(32, 128, C).transpose(1, 0, 2)  # [128, 32, C]

    r1 = bt2 @ w1 + b1  # pre-relu bias row [4C]
    r1c = np.ascontiguousarray(r1.reshape(32, 128).T, dtype=f32)  # [128, 32]

    rows = np.stack([b_proj, b2], 0)[None]  # [1, 2, C]

    ident = np.eye(128, dtype=f32)
    maskm = np.triu(np.full((128, 128), NEG, f32), 1)  # lhsT[t,s]=-1e9 iff s>t
    onesr = np.ones((1, 256), f32)

    shared = {
        "wa": np.ascontiguousarray(wa).astype(bf16),
        "w1": np.ascontiguousarray(w1p).astype(bf16),
        "w2": np.ascontiguousarray(w2p).astype(bf16),
        "ident": ident.astype(bf16),
        "maskm": maskm.astype(bf16),
        "onesr": onesr.astype(bf16),
        "rows": np.ascontiguousarray(rows).astype(bf16),
        "r1c": r1c,
    }
    if qkv_bias:
        qkr = np.stack([rq, rk], 0).reshape(2, 8, 128)[None]  # [1, {q,k}, blk, 128]
        shared["qkr"] = np.ascontiguousarray(qkr).astype(bf16)
    if v_bias:
        shared["vr"] = rv[None].astype(bf16)
    return shared, qkv_bias, v_bias


_CACHE = {}


def _get_program(bpc, qkv_bias, v_bias):
    key = (bpc, qkv_bias, v_bias)
    if key not in _CACHE:
        _CACHE[key] = build_program(bpc, qkv_bias, v_bias)
    return _CACHE[key]


def run(inputs, trace=False):
    from concourse.bass_utils import run_bass_kernel_spmd

    x = np.asarray(inputs["x"], np.float32)
    shared, qkv_bias, v_bias = prep_host(inputs)
    nc = _get_program(BPC, qkv_bias, v_bias)
    in_maps = []
    for i in range(NCORES):
        m = dict(shared)
        m["xin"] = np.ascontiguousarray(x[i * BPC:(i + 1) * BPC])
        in_maps.append(m)
    res = run_bass_kernel_spmd(
        nc, in_maps, core_ids=list(range(NCORES)), trace=trace
    )
    out = np.concatenate(
        [np.asarray(res.results[i]["yout"], np.float32) for i in range(NCORES)], 0
    )
    return out, res


def kernel(**inputs):
    out, _ = run(inputs, trace=False)
    return out


if __name__ == "__main__":
    nc = build_program(int(sys.argv[1]) if len(sys.argv) > 1 else 2)
    print("build ok")



# revision 27
# speedup vs baseline: 1.2743x; 1.2743x over previous
"""Trainium2 Bass kernel for a pre-norm transformer block (attention + MLP).

Contract: kernel(**inputs) takes the FULL unsharded inputs of
nn_Block_33775622815825 and returns the FULL output. The batch (B=128) is
sharded data-parallel across 8 NeuronCores (16 per core); the whole block
runs per-core with no collectives. All matmuls are bf16 with fp32 PSUM
accumulation (measured scale-relative error ~1.2e-3 vs the fp32 reference).

Per core, per batch element b (two-phase so bf16 weights fit in SBUF):
  phase A:  x -> LN1 -> h(bf16) -> hT chunks (batched DMA-xbar transpose)
            Q^T/K^T = Wqk^T @ hT  (two batch elems share each matmul, N=256)
            V = hT^T @ Wv          (interleaved into the scores pass below
                                    as PE filler while exps drain)
            per head pair: scoresT[s,t] = K_h Q_h^T/32 + (-1e9)*causal (PSUM)
                           weiT = exp(scoresT)      (one ACT op, bf16 out)
            per 4 heads:   attn_un[t,d] = weiT^T @ V_h   } same PSUM tile,
                           rowsum[t]    = weiT^T @ ones  } fused rowsum col
                           attn = attn_un * (1/rowsum)   (DVE, recip+bcast)
                           attnT block <- DMA-xbar transpose (proj layout)
            x1 = x + attnT^T @ Wproj + b_proj  (bias via rank-1 matmul,
                 residual via DRAM->DRAM seed + SWDGE accumulate-DMA)
  phase B:  x1 -> LN2 -> h2T
            hidT = relu(W1^T @ h2T + r1)   (bias fused into ACT relu drain)
            out  = x1 + hidT^T @ W2 + b2

Key tricks: LN gammas folded into weights host-side; LN betas / linear
biases become rank-1 matmul accumulations or per-partition ACT biases
(emitted only when nonzero); softmax skips max-subtraction (scores are
tiny); the causal mask is a -1e9 matmul into the scores PSUM; rstd is
exp(-0.5*ln(var+eps)) and a custom Bacc pass pins every activation to the
single natural_log_exp_and_others table set (one ACT_TABLE_LOAD total);
emission order software-pipelines the frontend one pair ahead and
interleaves next-batch work into the softmax tails to keep the in-order
PE queue fed.
"""
import os
import sys

import numpy as np

for _p in ("/opt/trn_rl_repo",):
    if _p not in sys.path:
        sys.path.insert(0, _p)

import ml_dtypes

import concourse.bass as bass
import concourse.tile as tile
from concourse import bacc
from concourse import mybir
from concourse.bass import ds, ts

F32 = mybir.dt.float32
BF16 = mybir.dt.bfloat16
AF = mybir.ActivationFunctionType
ALU = mybir.AluOpType

B, T, C, H, D = 128, 128, 1024, 16, 64
C4 = 4 * C
NCORES = 8
BPC = B // NCORES  # batch elements per core
EPS = 1e-5
SCL = float(C) ** -0.5  # softmax scale (1/32)
NEG = -1.0e9

bf16 = ml_dtypes.bfloat16


class _Bacc(bacc.Bacc):
    """Bacc whose activation-table pass resolves every function we use to
    the single `natural_log_exp_and_others` set (exp+ln+relu+copy live
    there together), so the kernel pays exactly one ACT_TABLE_LOAD instead
    of thrashing between exp_and_others and natural_log_exp_and_others."""

    _ONE_SET = "natural_log_exp_and_others"
    _OURS = {AF.Exp, AF.Ln, AF.Copy, AF.Relu, AF.Identity, AF.Square}

    def insert_act_table_loads(self):
        import bass_rust as _br

        from concourse.hw_specs import get_activation_tables

        has_activation = any(
            isinstance(i, mybir.InstActivation)
            for b in self.main_func.blocks
            for i in b.instructions
        )
        if not has_activation:
            return
        tables = []
        for name, funcs in get_activation_tables(self.m.arch).items():
            if name != self._ONE_SET:
                funcs = set(funcs) - self._OURS
            tables.append((name, funcs))
        _br.insert_act_table_loads(self, tables)


def build_program(bpc=BPC, qkv_bias=False, v_bias=False):
    """Emit the Bass/Tile program for one core handling `bpc` batch elems."""
    assert bpc % 2 == 0
    nc = _Bacc()

    xin = nc.declare_dram_parameter("xin", [bpc, T, C], F32, isOutput=False)
    # [c_part, {wq,wk,wv,wproj}, chunk, 1024] ; wproj is [hd_part, chunk, c]
    wa = nc.declare_dram_parameter("wa", [128, 4, 8, 1024], BF16, isOutput=False)
    w1 = nc.declare_dram_parameter("w1", [128, 8, C4], BF16, isOutput=False)
    w2 = nc.declare_dram_parameter("w2", [128, 32, C], BF16, isOutput=False)
    ident = nc.declare_dram_parameter("ident", [128, 128], BF16, isOutput=False)
    maskm = nc.declare_dram_parameter("maskm", [128, 128], BF16, isOutput=False)
    onesr = nc.declare_dram_parameter("onesr", [1, 256], BF16, isOutput=False)
    rows = nc.declare_dram_parameter("rows", [1, 2, C], BF16, isOutput=False)
    r1c = nc.declare_dram_parameter("r1c", [128, 32], F32, isOutput=False)
    if qkv_bias:
        qkr = nc.declare_dram_parameter("qkr", [1, 2, 8, 128], BF16, isOutput=False)
    if v_bias:
        vr = nc.declare_dram_parameter("vr", [1, C], BF16, isOutput=False)
    yout = nc.declare_dram_parameter("yout", [bpc, T, C], F32, isOutput=True)
    x1d = nc.dram_tensor("x1d", [bpc, T, C], F32)

    xin, wa, w1, w2 = xin[:], wa[:], w1[:], w2[:]
    ident_d, maskm_d, onesr_d, rows_d, r1c_d = (
        ident[:], maskm[:], onesr[:], rows[:], r1c[:],
    )
    yout, x1d = yout[:], x1d[:]

    npairs = bpc // 2

    with tile.TileContext(nc) as tc:
        from contextlib import ExitStack

        with ExitStack() as ctx:
            consts = ctx.enter_context(tc.tile_pool(name="consts", bufs=1))
            wpool = ctx.enter_context(tc.tile_pool(name="wpool", bufs=1))
            w1pool = ctx.enter_context(tc.tile_pool(name="w1pool", bufs=1))
            xpool = ctx.enter_context(tc.tile_pool(name="xpool", bufs=6))
            hpool = ctx.enter_context(tc.tile_pool(name="hpool", bufs=2))
            htpool = ctx.enter_context(tc.tile_pool(name="htpool", bufs=2))
            # x loads run two pairs ahead of use: any DMA-completion wait
            # the sem-lowering hoists into an earlier queue slot is then
            # referencing a transfer that finished a full pair ago

            respool = ctx.enter_context(tc.tile_pool(name="respool", bufs=2))
            stats = ctx.enter_context(tc.tile_pool(name="stats", bufs=4))
            # PSUM: acc (QKV / mlp1 accumulations), scatt (scores + attnT),
            # wide (proj / mlp2 outputs). 2 + 4 + 2 = 8 banks.
            psacc = ctx.enter_context(
                tc.tile_pool(name="psacc", bufs=2, space="PSUM")
            )


            # constants
            ident_sb = consts.tile([128, 128], BF16)
            nc.gpsimd.dma_start(out=ident_sb, in_=ident_d)
            maskm_sb = consts.tile([128, 128], BF16)
            nc.gpsimd.dma_start(out=maskm_sb, in_=maskm_d)
            ones_sb = consts.tile([1, 256], BF16)
            nc.gpsimd.dma_start(out=ones_sb, in_=onesr_d)
            rows_sb = consts.tile([1, 2, C], BF16)
            nc.gpsimd.dma_start(out=rows_sb, in_=rows_d)
            r1_sb = consts.tile([128, 32], F32)
            nc.gpsimd.dma_start(out=r1_sb, in_=r1c_d)
            eps_sb = consts.tile([128, 1], F32)
            nc.vector.memset(eps_sb, EPS)
            ones_col = consts.tile([128, 1], BF16)
            nc.vector.memset(ones_col, 1.0)
            if qkv_bias:
                qkr_sb = consts.tile([1, 2, 8, 128], BF16)
                nc.gpsimd.dma_start(out=qkr_sb, in_=qkr[:])
            if v_bias:
                vr_sb = consts.tile([1, C], BF16)
                nc.gpsimd.dma_start(out=vr_sb, in_=vr[:])

            waA = wpool.tile([128, 4, 8, 1024], BF16, tag="wbig")
            w1t = w1pool.tile([128, 8, C4], BF16)

            def layernorm(x_sb, tagp):
                """LN stats on [128, C] fp32; returns bf16 normalized tile."""
                st6 = stats.tile([128, 2, 6], F32, tag="st6" + tagp)
                nc.vector.bn_stats(out=st6[:, 0], in_=x_sb[:, 0:512])
                nc.vector.bn_stats(out=st6[:, 1], in_=x_sb[:, 512:1024])
                mv = stats.tile([128, 2], F32, tag="mv" + tagp)
                nc.vector.bn_aggr(out=mv, in_=st6)
                # rstd = exp(-0.5*ln(var+eps)) : stays in the exp/ln table set
                lnv = stats.tile([128, 1], F32, tag="lnv" + tagp)
                nc.scalar.activation(out=lnv, in_=mv[:, 1:2], func=AF.Ln, bias=eps_sb)
                rstd = stats.tile([128, 1], F32, tag="rstd" + tagp)
                nc.scalar.activation(out=rstd, in_=lnv, func=AF.Exp, scale=-0.5)
                # bounce rstd through DVE so the (wait-slot-limited)
                # tensor_scalar below never carries a cross-engine wait
                rstd2 = stats.tile([128, 1], F32, tag="rstd2" + tagp)
                nc.vector.tensor_copy(out=rstd2, in_=rstd)
                h_sb = hpool.tile([128, C], BF16, tag="h")
                nc.vector.tensor_scalar(
                    out=h_sb, in0=x_sb, scalar1=mv[:, 0:1], scalar2=rstd2,
                    op0=ALU.subtract, op1=ALU.mult,
                )
                return h_sb

            xlive = {}

            def load_x(b, src):
                # bulk loads ride the Pool/SWDGE queue: its rings are
                # separate from the HWDGE rings the latency-critical xbar
                # transposes use, so they never queue behind a bulk copy
                x_sb = xpool.tile([128, C], F32, tag="x", name=f"x_{b}")
                nc.gpsimd.dma_start(out=x_sb, in_=src[b])
                xlive[b] = x_sb

            def ln_transpose(b, hT, pi):
                h_sb = layernorm(xlive[b], "a")
                nc.sync.dma_start(out=hT[:, pi], in_=h_sb, transpose=True)

            def residual_store(b, psum, dst):
                # residual add in SBUF (DVE drains the PSUM) + plain store:
                # no DRAM->DRAM seed, no read-modify-write accumulate DMA
                o_sb = respool.tile([128, C], F32, tag="res", name=f"res_{b}")
                nc.vector.tensor_add(out=o_sb, in0=psum, in1=xlive.pop(b))
                nc.gpsimd.dma_start(out=dst[b], in_=o_sb)

            # ---------------- phase A ----------------
            with ExitStack() as actx:
                psatt = actx.enter_context(
                    tc.tile_pool(name="psatt", bufs=2, space="PSUM")
                )
                pswide = actx.enter_context(
                    tc.tile_pool(name="pswide", bufs=2, space="PSUM")
                )  # half-width [128,512] tiles: 2 bufs x 1 bank
                qkpool = actx.enter_context(tc.tile_pool(name="qkpool", bufs=2))
                vpool = actx.enter_context(tc.tile_pool(name="vpool", bufs=2))
                apool = actx.enter_context(tc.tile_pool(name="apool", bufs=2))
                weipool = actx.enter_context(tc.tile_pool(name="weipool", bufs=4))

                def frontA(pair):
                    hT = htpool.tile([128, 2, 8, 128], BF16, tag="ht",
                                     name=f"hT_{pair}")
                    for pi in range(2):
                        ln_transpose(2 * pair + pi, hT, pi)
                    return hT

                # first x loads go ahead of the weight slabs so the first
                # pair's LN starts immediately; wq/wk ride the ACT HWDGE
                # queue split in halves to smooth the DMA pipe
                for b in range(min(4, bpc)):
                    load_x(b, xin)
                for wi in (0, 1):
                    for hf in range(2):
                        nc.scalar.dma_start(
                            out=waA[:, wi, ts(hf, 4)], in_=wa[:, wi, ts(hf, 4)]
                        )
                front_next = frontA(0)
                for wi in range(2, 4):
                    nc.gpsimd.dma_start(out=waA[:, wi], in_=wa[:, wi])

                for pair in range(npairs):
                    hT = front_next
                    for pi in range(2):
                        b = 2 * (pair + 2) + pi
                        if b < bpc:
                            load_x(b, xin)
                    if pair + 1 < npairs:
                        front_next = frontA(pair + 1)
                    # stream w1 slabs during phase A
                    if pair < 8:
                        nc.gpsimd.dma_start(out=w1t[:, pair], in_=w1[:, pair])
                    if pair == npairs - 1:
                        for s in range(min(npairs, 8), 8):
                            nc.gpsimd.dma_start(out=w1t[:, s], in_=w1[:, s])

                    # Q^T / K^T : [hd', 2b, t] accumulated over c-chunks
                    qt_sb = qkpool.tile([128, 2, 8, 128], BF16, tag="qt")
                    kt_sb = qkpool.tile([128, 2, 8, 128], BF16, tag="kt")
                    for wi, dst in ((0, qt_sb), (1, kt_sb)):
                        for blk in range(8):
                            ps = psacc.tile([128, 2, 128], F32, tag="ps")
                            if qkv_bias:
                                nc.tensor.matmul(
                                    ps, lhsT=qkr_sb[0:1, wi, blk],
                                    rhs=ones_sb,
                                    start=True, stop=False,
                                )
                            for cc in range(8):
                                nc.tensor.matmul(
                                    ps,
                                    lhsT=waA[:, wi, cc, ts(blk, 128)],
                                    rhs=hT[:, :, cc],
                                    start=(cc == 0 and not qkv_bias),
                                    stop=(cc == 7),
                                )
                            nc.scalar.copy(out=dst[:, :, blk], in_=ps)
                    state = {}

                    def sc_pass(pi):
                        b = 2 * pair + pi
                        v_sb = vpool.tile([128, C], BF16, tag="v")

                        def v_half(half, pi=pi, v_sb=v_sb):
                            psv = psacc.tile([128, 512], F32, tag="ps")
                            if v_bias:
                                nc.tensor.matmul(
                                    psv, lhsT=ones_sb[0:1, 0:128],
                                    rhs=vr_sb[0:1, ds(half * 512, 512)],
                                    start=True, stop=False,
                                )
                            for cc in range(8):
                                nc.tensor.matmul(
                                    psv,
                                    lhsT=hT[:, pi, cc],
                                    rhs=waA[:, 2, cc, ds(half * 512, 512)],
                                    start=(cc == 0 and not v_bias),
                                    stop=(cc == 7),
                                )
                            nc.scalar.copy(
                                out=v_sb[:, ds(half * 512, 512)], in_=psv
                            )

                        # pass 1: transposed scores exp(K Q^T / 32) for all 16
                        # heads -- the exp output IS the attn matmul's
                        # stationary operand, so no wei transpose is needed.
                        # V matmuls interleave as PE filler.
                        weiTs = []
                        for blk in range(8):
                            weiT_un = weipool.tile(
                                [128, 2, 128], BF16, tag="weiT", bufs=9
                            )
                            # two heads' scoresT share one PSUM bank; one exp
                            sc2 = psatt.tile([128, 2, 128], F32, tag="ps")
                            for sub in range(2):
                                po = sub * 64
                                nc.tensor.matmul(
                                    sc2[:, sub], lhsT=kt_sb[po:po + 64, pi, blk],
                                    rhs=qt_sb[po:po + 64, pi, blk],
                                    start=True, stop=False,
                                )
                                nc.tensor.matmul(
                                    sc2[:, sub], lhsT=maskm_sb, rhs=ident_sb,
                                    start=False, stop=True,
                                )
                            nc.scalar.activation(
                                out=weiT_un, in_=sc2, func=AF.Exp, scale=SCL,
                            )
                            weiTs.append(weiT_un)
                            if blk == 2:
                                v_half(0)
                            elif blk == 5:
                                v_half(1)
                        state[pi] = (v_sb, weiTs)

                    def attnT_pass(pi):
                        v_sb, weiTs = state[pi]
                        attnT = apool.tile([128, 8, 128], BF16, tag="attnT")
                        for grp in range(4):  # 4 heads per group
                            # attn_un [t, head, d] plus a fused rowsum column
                            att_ps = psatt.tile([128, 4, 65], F32, tag="att",
                                                bufs=2)
                            for j in range(4):
                                hh = 4 * grp + j
                                blk, sub = hh // 2, hh % 2
                                nc.tensor.matmul(
                                    att_ps[:, j, 0:64],
                                    lhsT=weiTs[blk][:, sub],
                                    rhs=v_sb[:, ds(hh * 64, 64)],
                                    start=True, stop=True,
                                )
                                nc.tensor.matmul(
                                    att_ps[:, j, 64:65],
                                    lhsT=weiTs[blk][:, sub],
                                    rhs=ones_col,
                                    start=True, stop=True,
                                )
                            rr4 = stats.tile([128, 4], F32, tag="rr")
                            nc.vector.reciprocal(out=rr4, in_=att_ps[:, :, 64])
                            attn_bf = weipool.tile([128, 4, 64], BF16,
                                                   tag="anorm", bufs=2)
                            nc.vector.tensor_mul(
                                out=attn_bf, in0=att_ps[:, :, 0:64],
                                in1=rr4.to_broadcast([128, 4, 64]),
                            )
                            # blockwise xbar transpose writes attnT directly
                            # in the proj lhsT layout [hd', t]
                            nc.sync.dma_start(
                                out=attnT[:, ts(grp, 2)], in_=attn_bf,
                                transpose=True,
                            )
                        state[pi] = attnT

                    def proj_pass(pi):
                        attnT = state[pi]
                        b = 2 * pair + pi
                        x_sb = xlive.pop(b)
                        o_sb = respool.tile([128, C], F32, tag="res",
                                            name=f"res_{b}")
                        for half in range(2):
                            prp = pswide.tile([128, 512], F32, tag="wide")
                            nc.tensor.matmul(
                                prp,
                                lhsT=ones_sb[0:1, 0:128],
                                rhs=rows_sb[0:1, 0, ds(half * 512, 512)],
                                start=True, stop=False,
                            )
                            for hc in range(8):
                                nc.tensor.matmul(
                                    prp,
                                    lhsT=attnT[:, hc],
                                    rhs=waA[:, 3, hc, ds(half * 512, 512)],
                                    start=False, stop=(hc == 7),
                                )
                            nc.vector.tensor_add(
                                out=o_sb[:, ds(half * 512, 512)], in0=prp,
                                in1=x_sb[:, ds(half * 512, 512)],
                            )
                        nc.gpsimd.dma_start(out=x1d[b], in_=o_sb)

                    # emission order keeps PE fed through the b1 softmax
                    # tail: b0's proj fills the gap before b1's attnT
                    sc_pass(0)
                    attnT_pass(0)
                    sc_pass(1)
                    proj_pass(0)
                    attnT_pass(1)
                    proj_pass(1)

            # ---------------- phase B ----------------
            w2t = wpool.tile([128, 32, C], BF16, tag="wbig")

            with tc.tile_pool(name="hidpool", bufs=1) as hidpool, \
                    tc.tile_pool(name="pswideB", bufs=2,
                                 space="PSUM") as pswideB:

                def frontB(pair):
                    h2T = htpool.tile([128, 2, 8, 128], BF16, tag="ht",
                                      name=f"h2T_{pair}")
                    for pi in range(2):
                        ln_transpose(2 * pair + pi, h2T, pi)
                    return h2T

                # x1 loads for the first two pairs go ahead of the w2
                # slabs; pair 0's mlp1 covers the w2 transfer
                for b in range(min(4, bpc)):
                    load_x(b, x1d)
                front_next = frontB(0)
                for mc in range(4):
                    nc.gpsimd.dma_start(out=w2t[:, ts(mc, 4)],
                                        in_=w2[:, ts(mc, 4)])
                for pair in range(npairs):
                    h2T = front_next
                    for pi in range(2):
                        b = 2 * (pair + 2) + pi
                        if b < bpc:
                            load_x(b, x1d)
                    if pair + 1 < npairs:
                        front_next = frontB(pair + 1)
                    if pair == 0:
                        for mc in range(4, 8):
                            nc.gpsimd.dma_start(out=w2t[:, ts(mc, 4)],
                                                in_=w2[:, ts(mc, 4)])
                    hid = hidpool.tile([128, 2, 32, 128], BF16, tag="hid")
                    for mb in range(32):
                        ps1 = psacc.tile([128, 2, 128], F32, tag="ps")
                        for cc in range(8):
                            nc.tensor.matmul(
                                ps1, lhsT=w1t[:, cc, ts(mb, 128)],
                                rhs=h2T[:, :, cc],
                                start=(cc == 0), stop=(cc == 7),
                            )
                        nc.scalar.activation(
                            out=hid[:, :, mb], in_=ps1, func=AF.Relu,
                            bias=r1_sb[:, mb:mb + 1],
                        )
                    for pi in range(2):
                        b = 2 * pair + pi
                        ps2 = pswideB.tile([128, C], F32, tag="wideB")
                        for half in range(2):
                            nc.tensor.matmul(
                                ps2[:, ds(half * 512, 512)],
                                lhsT=ones_sb[0:1, 0:128],
                                rhs=rows_sb[0:1, 1, ds(half * 512, 512)],
                                start=True, stop=False,
                            )
                            for mc in range(32):
                                nc.tensor.matmul(
                                    ps2[:, ds(half * 512, 512)],
                                    lhsT=hid[:, pi, mc],
                                    rhs=w2t[:, mc, ds(half * 512, 512)],
                                    start=False, stop=(mc == 31),
                                )
                        residual_store(b, ps2, yout)

    # lower to HW-legal IR: split >1-wait instructions into EventSemaphore
    # preludes, move matmul waits onto ldweights, alloc regs, act tables
    nc.compile()
    return nc


def prep_host(inputs):
    """Host-side weight packing / folding. Returns (shared in_map, flags)."""
    f32 = np.float32
    wq = np.asarray(inputs["wq"], f32)
    wk = np.asarray(inputs["wk"], f32)
    wv = np.asarray(inputs["wv"], f32)
    w_proj = np.asarray(inputs["w_proj"], f32)
    b_proj = np.asarray(inputs["b_proj"], f32)
    w1 = np.asarray(inputs["w1"], f32)
    b1 = np.asarray(inputs["b1"], f32)
    w2 = np.asarray(inputs["w2"], f32)
    b2 = np.asarray(inputs["b2"], f32)
    g1 = np.asarray(inputs["ln1_g"], f32)
    bt1 = np.asarray(inputs["ln1_b"], f32)
    g2 = np.asarray(inputs["ln2_g"], f32)
    bt2 = np.asarray(inputs["ln2_b"], f32)

    wq_f = wq.transpose(1, 0, 2).reshape(C, C)  # [c, h*d]
    wk_f = wk.transpose(1, 0, 2).reshape(C, C)
    wv_f = wv.transpose(1, 0, 2).reshape(C, C)

    # fold LN1 gamma into qkv weights; LN1 beta becomes rank-1 rows
    rq = bt1 @ wq_f
    rk = bt1 @ wk_f
    rv = bt1 @ wv_f
    qkv_bias = bool(np.abs(rq).max() > 0 or np.abs(rk).max() > 0)
    v_bias = bool(np.abs(rv).max() > 0)

    wa = np.stack(
        [g1[:, None] * wq_f, g1[:, None] * wk_f, g1[:, None] * wv_f, w_proj], 0
    )  # [4, 1024, 1024]
    wa = wa.reshape(4, 8, 128, 1024).transpose(2, 0, 1, 3)  # [128, 4, 8, 1024]

    w1_eff = g2[:, None] * w1  # [C, 4C]
    w1p = w1_eff.reshape(8, 128, C4).transpose(1, 0, 2)  # [128, 8, 4C]
    w2p = w2.reshape(32, 128, C).transpose(1, 0, 2)  # [128, 32, C]

    r1 = bt2 @ w1 + b1  # pre-relu bias row [4C]
    r1c = np.ascontiguousarray(r1.reshape(32, 128).T, dtype=f32)  # [128, 32]

    rows = np.stack([b_proj, b2], 0)[None]  # [1, 2, C]

    ident = np.eye(128, dtype=f32)
    maskm = np.triu(np.full((128, 128), NEG, f32), 1)  # lhsT[t,s]=-1e9 iff s>t
    onesr = np.ones((1, 256), f32)

    shared = {
        "wa": np.ascontiguousarray(wa).astype(bf16),
        "w1": np.ascontiguousarray(w1p).astype(bf16),
        "w2": np.ascontiguousarray(w2p).astype(bf16),
        "ident": ident.astype(bf16),
        "maskm": maskm.astype(bf16),
        "onesr": onesr.astype(bf16),
        "rows": np.ascontiguousarray(rows).astype(bf16),
        "r1c": r1c,
    }
    if qkv_bias:
        qkr = np.stack([rq, rk], 0).reshape(2, 8, 128)[None]  # [1, {q,k}, blk, 128]
        shared["qkr"] = np.ascontiguousarray(qkr).astype(bf16)
    if v_bias:
        shared["vr"] = rv[None].astype(bf16)
    return shared, qkv_bias, v_bias


_CACHE = {}


def _get_program(bpc, qkv_bias, v_bias):
    key = (bpc, qkv_bias, v_bias)
    if key not in _CACHE:
        _CACHE[key] = build_program(bpc, qkv_bias, v_bias)
    return _CACHE[key]


def run(inputs, trace=False):
    from concourse.bass_utils import run_bass_kernel_spmd

    x = np.asarray(inputs["x"], np.float32)
    shared, qkv_bias, v_bias = prep_host(inputs)
    nc = _get_program(BPC, qkv_bias, v_bias)
    in_maps = []
    for i in range(NCORES):
        m = dict(shared)
        m["xin"] = np.ascontiguousarray(x[i * BPC:(i + 1) * BPC])
        in_maps.append(m)
    res = run_bass_kernel_spmd(
        nc, in_maps, core_ids=list(range(NCORES)), trace=trace
    )
    out = np.concatenate(
        [np.asarray(res.results[i]["yout"], np.float32) for i in range(NCORES)], 0
    )
    return out, res


def kernel(**inputs):
    out, _ = run(inputs, trace=False)
    return out


if __name__ == "__main__":
    nc = build_program(int(sys.argv[1]) if len(sys.argv) > 1 else 2)
    print("build ok")

